# revision 12
# baseline (speedup 1.0000x reference)
"""Conv2d 3x3 VALID (NHWC x HWIO -> NHWC) on 8 Trainium2 NeuronCores.

Strategy ("dual"): data-parallel over batch (2 images/core), and within a
core the two images run concurrently on the two 64-row tiles of the PE
array (64x128 row-tiling mode):

  - SBUF partitions 0:64  hold image 0's 64 channels (flat H*W signal)
  - SBUF partitions 64:128 hold image 1's 64 channels
  - PE tile T0 (rows 0:64)  computes image 0, tile T8 (rows 64:128) image 1

Per 512-position window each image needs 9 K=64 matmuls (one per conv tap,
each just a different column offset into the same SBUF slab - no shifted
data copies), accumulated in that image's own PSUM bank.  The two tiles run
concurrently, so the effective cost is 9 x 512 cycles per *two* windows =
4.5 K=128-equivalent matmuls per window (the dense-packing floor).

Outputs at flat positions whose column lands in {W-2, W-1} or row in
{H-2, H-1} are garbage and are sliced away host-side.

Self-contained: hardcodes shapes from the problem spec
  x: (16, 224, 224, 64) f32, w: (3, 3, 64, 128) f32 -> y: (16, 222, 222, 128).
"""
import contextlib
import os
import numpy as np

import concourse.bacc as bacc
import concourse.mybir as mybir
from concourse.tile import TileContext
from concourse.bass_utils import run_bass_kernel_spmd

N_CORES = 8
N_IMG = 2          # images per core
H = W = 224
CIN, COUT = 64, 128
L = H * W          # 50176 flat positions per image
WIN = 512          # window width (one fp32 PSUM bank)
MARGIN = 2 * W + 4
XT2_W = L + WIN    # zero-padded per-image input width

VARIANT = os.environ.get("CONV_VARIANT", "dual")
OUT_DT = os.environ.get("CONV_OUT_DT", "f16")
IN_DT = os.environ.get("CONV_IN_DT", "f16")
S = int(os.environ.get("CONV_S", "7168"))        # slab positions (dual)
WPAIR = int(os.environ.get("CONV_WPAIR", "2"))   # windows per tap-group
A_BUFS = int(os.environ.get("CONV_A_BUFS", "3"))
PS_BUFS = int(os.environ.get("CONV_PS_BUFS", "8"))
O_BUFS = int(os.environ.get("CONV_O_BUFS", "3"))
OGRAN = int(os.environ.get("CONV_OGRAN", "14"))  # windows per output DMA
EVAC = os.environ.get("CONV_EVAC", "vs")         # v, s, or vs (alternate)
ELIDE = os.environ.get("CONV_ELIDE", "1") == "1"
TRIM = os.environ.get("CONV_TRIM", "1") == "1"
PSMERGE = os.environ.get("CONV_PSMERGE", "1") == "1"
# last window holding any valid output: flat pos 49664..49725 -> N=64 suffices
LAST_WIN = (H * (H - 2)) // WIN  # 97
LAST_N = 64

TAPS = [(r, s) for r in range(3) for s in range(3)]


def elide_redundant_ldweights(nc):
    """Remove InstLdweights that reload the identical weights already resident
    in the same PE row-group (tile_position). Run before nc.compile().

    Consecutive matmuls that reuse the same stationary operand each get a 1:1
    InstLdweights from the bass lowering; the PE keeps per-row-group weights,
    so a reload of the same AP is a no-op that still costs issue slots and
    weight-bus bandwidth. Only elides LDWs with no sync waits/updates; resets
    tracking at any other PE instruction and at block boundaries.
    """
    n_del = 0
    for blk in nc.main_func.blocks:
        cur = {}  # tile_position -> weights key
        keep = []
        for inst in blk.instructions:
            if isinstance(inst, mybir.InstLdweights):
                a = inst.ins[0]
                key = (a.memref, a.offset, str(a.ap), str(a.dtype),
                       inst.perf_mode, inst.is_transpose)
                tp = inst.tile_position
                si = inst.sync_info
                clean = si is None or (not si.on_wait and not si.on_update)
                if clean and cur.get(tp) == key:
                    n_del += 1
                    continue  # drop redundant reload
                cur[tp] = key
            elif isinstance(inst, mybir.InstMatmult):
                pass  # matmuls don't disturb loaded weights of other groups
            elif getattr(inst, "engine", None) == mybir.EngineType.PE:
                cur = {}  # conservative: unknown PE inst invalidates tracking
            keep.append(inst)
        blk.instructions[:] = keep
    return n_del


def np_in_dt(in_dt=None):
    in_dt = in_dt or IN_DT
    if in_dt == "f16":
        return np.float16
    if in_dt == "bf16":
        import ml_dtypes
        return np.dtype(ml_dtypes.bfloat16)
    return np.float32


def my_in_dt(in_dt=None):
    in_dt = in_dt or IN_DT
    return {"f32r": mybir.dt.float32r, "f16": mybir.dt.float16,
            "bf16": mybir.dt.bfloat16}[in_dt]


# ---------------------------------------------------------------- dual ----

def build_dual(out_dt=OUT_DT, s_pos=None, a_bufs=A_BUFS, ps_bufs=PS_BUFS,
               o_bufs=O_BUFS, repeat=1, in_dt=IN_DT, wpair=None,
               ogran=None, evac=None, no_out=False, no_evac=False, tiles=2,
               static_in=False, m_cols=COUT, order="seq", elide=ELIDE,
               trim=TRIM, psmerge=PSMERGE):
    if psmerge and ps_bufs > 2:
        ps_bufs = 2  # two 2-bank tiles per tag x two tags = all 8 PSUM banks
    s_pos = s_pos or S
    wpair = wpair or WPAIR
    ogran = ogran or OGRAN
    evac = evac or EVAC
    f32 = mybir.dt.float32
    idt = my_in_dt(in_dt)
    out_mydt = f32 if out_dt == "f32" else mybir.dt.float16

    assert L % s_pos == 0 and s_pos % WIN == 0
    n_slabs = L // s_pos
    n_win = s_pos // WIN           # windows per slab (per image)
    assert n_win % ogran == 0

    nc = bacc.Bacc("TRN2", target_bir_lowering=False, debug=False)
    # rows 0:64 image0 channels, rows 64:128 image1 channels
    xt2 = nc.declare_dram_parameter("xt2", [128, XT2_W], idt, isOutput=False)
    # tap j weights replicated on both partition halves
    wt2 = nc.declare_dram_parameter("wt2", [128, 9 * COUT], idt, isOutput=False)
    # cols 0:L image0, cols L:2L image1
    yt = nc.declare_dram_parameter("yt", [COUT, 2 * L], out_mydt, isOutput=True)

    with TileContext(nc) as tc:
        with (
            tc.tile_pool(name="wpool", bufs=1) as wpool,
            tc.tile_pool(name="apool", bufs=a_bufs) as apool,
            tc.tile_pool(name="opool", bufs=o_bufs) as opool,
            tc.tile_pool(name="pspool", bufs=ps_bufs, space="PSUM") as pspool,
        ):
            w_sb = wpool.tile([128, 9 * COUT], idt)
            nc.sync.dma_start(out=w_sb[:, :], in_=wt2[:, :])
            if static_in:
                xd_static = wpool.tile([128, s_pos + MARGIN], idt,
                                       name="xd_static")
                nc.sync.dma_start(out=xd_static[:, :],
                                  in_=xt2[:, 0:s_pos + MARGIN])

            loop_cm = tc.For_i(0, repeat, 1) if repeat > 1 \
                else contextlib.nullcontext()
            ev = 0
            with loop_cm:
              for si in range(n_slabs):
                base = si * s_pos
                if static_in:
                    xd = xd_static
                else:
                    xd = apool.tile([128, s_pos + MARGIN], idt, tag="xd",
                                    name=f"xd_{si}")
                    nc.sync.dma_start(out=xd[:, :],
                                      in_=xt2[:, base:base + s_pos + MARGIN])
                for og in range(0, n_win, ogran):
                    sts = [opool.tile([128, ogran * WIN], out_mydt,
                                      tag=f"st{t}", name=f"st{t}_{si}_{og}")
                           for t in range(2)]
                    for wp in range(og, og + ogran, wpair):
                        wn = min(wpair, og + ogran - wp)
                        if psmerge:
                            assert wn == wpair == 2
                            acc2 = [pspool.tile([128, 2 * WIN], f32,
                                                tag=f"acc2_{t}",
                                                name=f"acc2_{si}_{t}_{wp}")
                                    for t in range(2)]
                            accs = [[acc2[t][:, wi * WIN:(wi + 1) * WIN]
                                     for wi in range(wn)] for t in range(2)]
                        else:
                            accs = [[pspool.tile([128, WIN], f32, tag="acc",
                                                 name=f"acc_{si}_{t}_{wi}")
                                     for wi in range(wp, wp + wn)]
                                    for t in range(2)]
                        for j in range(9):
                            r, s = TAPS[j]
                            off = r * W + s
                            st_j = (j == 0)
                            sp_j = (j == 8)
                            if order == "alt":
                                tw = [(t, wi) for wi in range(wn)
                                      for t in range(tiles)]
                            else:
                                tw = [(t, wi) for t in range(tiles)
                                      for wi in range(wn)]
                            for t, wi in tw:
                                p0 = t * 64
                                q0 = (wp + wi) * WIN + off
                                n_mm = WIN
                                if trim and si * n_win + wp + wi == LAST_WIN:
                                    n_mm = LAST_N
                                nc.tensor.matmul(
                                    accs[t][wi][0:m_cols, 0:n_mm],
                                    w_sb[p0:p0 + 64,
                                         j * COUT:j * COUT + m_cols],
                                    xd[p0:p0 + 64, q0:q0 + n_mm],
                                    start=st_j, stop=sp_j,
                                )
                        # evacuate PSUM -> SBUF (cast), alternating engines
                        if no_evac:
                            continue
                        if psmerge:
                            pieces = [(t, acc2[t][:, :], 2 * WIN)
                                      for t in range(tiles)]
                        else:
                            pieces = [(t, accs[t][wi][:], WIN, wi)
                                      for t in range(tiles)
                                      for wi in range(wn)]
                        for piece in pieces:
                            if psmerge:
                                t, src, width = piece
                                c0 = (wp - og) * WIN
                            else:
                                t, src, width, wi = piece
                                c0 = (wp - og + wi) * WIN
                            dst = sts[t][:, c0:c0 + width]
                            if evac == "v" or (evac == "vs" and ev % 2 == 0):
                                nc.vector.tensor_copy(dst, src)
                            else:
                                nc.scalar.activation(
                                    dst, src,
                                    mybir.ActivationFunctionType.Copy)
                            ev += 1
                    if no_out or no_evac:
                        continue
                    for t in range(tiles):
                        col = t * L + base + og * WIN
                        nc.sync.dma_start(
                            out=yt[:, col:col + ogran * WIN],
                            in_=sts[t][:, :])
    if elide:
        elide_redundant_ldweights(nc)
    nc.compile()
    return nc


def prep_xt2(xs, in_dt=None):
    """xs: (2, H, W, 64) f32 -> (128, XT2_W): img0 chans on rows 0:64."""
    out = np.zeros((128, XT2_W), dtype=np_in_dt(in_dt))
    for t in range(N_IMG):
        flat = np.ascontiguousarray(xs[t].transpose(2, 0, 1)).reshape(CIN, L)
        out[t * CIN:(t + 1) * CIN, :L] = flat
    return out


def pack_wt2(w, in_dt=None):
    """w: (3,3,64,128) -> (128, 9*128), tap j on both partition halves."""
    wt = np.zeros((128, 9 * COUT), dtype=np_in_dt(in_dt))
    for j, (r, s) in enumerate(TAPS):
        wt[0:CIN, j * COUT:(j + 1) * COUT] = w[r, s]
        wt[CIN:128, j * COUT:(j + 1) * COUT] = w[r, s]
    return wt


def post_yt_dual(yt_arr):
    """(128, 2L) -> (2, 222, 222, 128) f32."""
    y = np.asarray(yt_arr, dtype=np.float32).reshape(COUT, N_IMG, H, W)
    y = y[:, :, :H - 2, :W - 2]
    return np.ascontiguousarray(y.transpose(1, 2, 3, 0))


# ------------------------------------------------------------ v2 (old) ----

V2_S = 4096
V2_XT_W = N_IMG * L + WIN


def make_plan_v2():
    return ([(0, r * W, 128, [(r, 0), (r, 1)]) for r in range(3)]
            + [(1, 2, 128, [(0, 2), (1, 2)]),
               (0, 2 * W + 2, 64, [(2, 2), None])])


def build_v2(out_dt=OUT_DT, s_pos=V2_S, a_bufs=A_BUFS, ps_bufs=PS_BUFS,
             o_bufs=8, repeat=1, in_dt=IN_DT):
    plan = make_plan_v2()
    n_mm = len(plan)
    Q = N_IMG * L
    f32 = mybir.dt.float32
    idt = my_in_dt(in_dt)
    out_mydt = f32 if out_dt == "f32" else mybir.dt.float16

    nc = bacc.Bacc("TRN2", target_bir_lowering=False, debug=False)
    xt = nc.declare_dram_parameter("xt", [CIN, V2_XT_W], idt, isOutput=False)
    wt = nc.declare_dram_parameter("wt", [n_mm, 128, COUT], idt, isOutput=False)
    yt = nc.declare_dram_parameter("yt", [COUT, Q], out_mydt, isOutput=True)

    with TileContext(nc) as tc:
        with (
            tc.tile_pool(name="wpool", bufs=1) as wpool,
            tc.tile_pool(name="apool", bufs=a_bufs) as apool,
            tc.tile_pool(name="opool", bufs=o_bufs) as opool,
            tc.tile_pool(name="pspool", bufs=ps_bufs, space="PSUM") as pspool,
        ):
            w_sb = wpool.tile([128, n_mm * COUT], idt)
            for i in range(n_mm):
                nc.sync.dma_start(out=w_sb[:, i * COUT:(i + 1) * COUT],
                                  in_=wt[i, :, :])

            n_slabs = (Q + s_pos - 1) // s_pos
            loop_cm = tc.For_i(0, repeat, 1) if repeat > 1 \
                else contextlib.nullcontext()
            with loop_cm:
              for si in range(n_slabs):
                base = si * s_pos
                sh = min(s_pos, Q - base)
                tiles = [apool.tile([128, s_pos + MARGIN], idt, tag=f"t{t}",
                                    name=f"tile{t}_{si}")
                         for t in range(2)]
                # all four halves straight from HBM (no SBUF->SBUF chains)
                nc.sync.dma_start(out=tiles[0][0:CIN, 0:sh + MARGIN],
                                  in_=xt[:, base:base + sh + MARGIN])
                nc.sync.dma_start(out=tiles[0][CIN:128, 0:2 * W + sh],
                                  in_=xt[:, base + 1:base + 1 + 2 * W + sh])
                nc.sync.dma_start(out=tiles[1][0:CIN, 0:sh + 2],
                                  in_=xt[:, base:base + sh + 2])
                nc.sync.dma_start(out=tiles[1][CIN:128, 0:sh + 2],
                                  in_=xt[:, base + W:base + W + sh + 2])

                for q0 in range(0, sh, WIN):
                    acc = pspool.tile([128, WIN], f32)
                    for j, (t, off, kk, _) in enumerate(plan):
                        nc.tensor.matmul(
                            acc[:],
                            w_sb[0:kk, j * COUT:(j + 1) * COUT],
                            tiles[t][0:kk, off + q0: off + q0 + WIN],
                            start=(j == 0),
                            stop=(j == n_mm - 1),
                        )
                    st = opool.tile([128, WIN], out_mydt)
                    nc.vector.tensor_copy(st[:], acc[:])
                    nc.sync.dma_start(out=yt[:, base + q0: base + q0 + WIN],
                                      in_=st[:])
    nc.compile()
    return nc


def pack_wt_v2(w, in_dt=None):
    plan = make_plan_v2()
    wt = np.zeros((len(plan), 128, COUT), dtype=np_in_dt(in_dt))
    for i, (_, _, _, taps) in enumerate(plan):
        (r0, s0), bot = taps
        wt[i, 0:CIN] = w[r0, s0]
        if bot is not None:
            r1, s1 = bot
            wt[i, CIN:128] = w[r1, s1]
    return wt


def prep_xt_v2(xs, in_dt=None):
    flat = np.ascontiguousarray(xs.transpose(3, 0, 1, 2)).reshape(CIN, N_IMG * L)
    out = np.zeros((CIN, V2_XT_W), dtype=np_in_dt(in_dt))
    out[:, :flat.shape[1]] = flat
    return out


def post_yt_v2(yt_arr):
    y = np.asarray(yt_arr, dtype=np.float32).reshape(COUT, N_IMG, H, W)
    y = y[:, :, :H - 2, :W - 2]
    return np.ascontiguousarray(y.transpose(1, 2, 3, 0))


# ------------------------------------------------------------ driver ------

def build_nc(repeat=1, variant=None):
    variant = variant or VARIANT
    if variant == "dual":
        return build_dual(repeat=repeat)
    return build_v2(repeat=repeat)


_NC_CACHE = {}


def _get_nc():
    key = (VARIANT, OUT_DT, IN_DT, S, WPAIR, A_BUFS, PS_BUFS, O_BUFS, OGRAN,
           EVAC)
    if key not in _NC_CACHE:
        _NC_CACHE[key] = build_nc()
    return _NC_CACHE[key]


def make_in_maps(x, w, variant=None):
    variant = variant or VARIANT
    if variant == "dual":
        wt = pack_wt2(w)
        return [{"xt2": prep_xt2(x[c * N_IMG:(c + 1) * N_IMG]), "wt2": wt}
                for c in range(N_CORES)]
    wt = pack_wt_v2(w)
    return [{"xt": prep_xt_v2(x[c * N_IMG:(c + 1) * N_IMG]), "wt": wt}
            for c in range(N_CORES)]


def kernel(x, w):
    x = np.asarray(x, dtype=np.float32)
    w = np.asarray(w, dtype=np.float32)
    nc = _get_nc()
    in_maps = make_in_maps(x, w)
    res = run_bass_kernel_spmd(nc, in_maps, list(range(N_CORES)))
    out = np.empty((N_CORES * N_IMG, H - 2, W - 2, COUT), dtype=np.float32)
    post = post_yt_dual if VARIANT == "dual" else post_yt_v2
    for c in range(N_CORES):
        out[c * N_IMG:(c + 1) * N_IMG] = post(res.results[c]["yt"])
    return out



# revision 16
# speedup vs baseline: 1.0135x; 1.0135x over previous
"""Conv2d 3x3 VALID (NHWC x HWIO -> NHWC) on 8 Trainium2 NeuronCores.

Strategy ("dual"): data-parallel over batch (2 images/core), and within a
core the two images run concurrently on the two 64-row tiles of the PE
array (64x128 row-tiling mode):

  - SBUF partitions 0:64  hold image 0's 64 channels (flat H*W signal)
  - SBUF partitions 64:128 hold image 1's 64 channels
  - PE tile T0 (rows 0:64)  computes image 0, tile T8 (rows 64:128) image 1

Per 512-position window each image needs 9 K=64 matmuls (one per conv tap,
each just a different column offset into the same SBUF slab - no shifted
data copies), accumulated in that image's own PSUM bank.  The two tiles run
concurrently, so the effective cost is 9 x 512 cycles per *two* windows =
4.5 K=128-equivalent matmuls per window (the dense-packing floor).

Outputs at flat positions whose column lands in {W-2, W-1} or row in
{H-2, H-1} are garbage and are sliced away host-side.

Self-contained: hardcodes shapes from the problem spec
  x: (16, 224, 224, 64) f32, w: (3, 3, 64, 128) f32 -> y: (16, 222, 222, 128).
"""
import contextlib
import os
import numpy as np

import concourse.bacc as bacc
import concourse.mybir as mybir
from concourse.tile import TileContext
from concourse.bass_utils import run_bass_kernel_spmd

N_CORES = 8
N_IMG = 2          # images per core
H = W = 224
CIN, COUT = 64, 128
L = H * W          # 50176 flat positions per image
WIN = 512          # window width (one fp32 PSUM bank)
MARGIN = 2 * W + 4
XT2_W = L + WIN    # zero-padded per-image input width

VARIANT = os.environ.get("CONV_VARIANT", "dual")
OUT_DT = os.environ.get("CONV_OUT_DT", "f16")
IN_DT = os.environ.get("CONV_IN_DT", "f16")
S = int(os.environ.get("CONV_S", "7168"))        # slab positions (dual)
WPAIR = int(os.environ.get("CONV_WPAIR", "2"))   # windows per tap-group
A_BUFS = int(os.environ.get("CONV_A_BUFS", "4"))
PS_BUFS = int(os.environ.get("CONV_PS_BUFS", "8"))
O_BUFS = int(os.environ.get("CONV_O_BUFS", "3"))
OGRAN = int(os.environ.get("CONV_OGRAN", "14"))  # windows per output DMA
EVAC = os.environ.get("CONV_EVAC", "vs")         # v, s, or vs (alternate)
ELIDE = os.environ.get("CONV_ELIDE", "1") == "1"
TRIM = os.environ.get("CONV_TRIM", "1") == "1"
PSMERGE = os.environ.get("CONV_PSMERGE", "1") == "1"
IN_CHUNKS = int(os.environ.get("CONV_INCH", "4"))
# last window holding any valid output: flat pos 49664..49725 -> N=64 suffices
LAST_WIN = (H * (H - 2)) // WIN  # 97
LAST_N = 64

TAPS = [(r, s) for r in range(3) for s in range(3)]


def elide_redundant_ldweights(nc):
    """Remove InstLdweights that reload the identical weights already resident
    in the same PE row-group (tile_position). Run before nc.compile().

    Consecutive matmuls that reuse the same stationary operand each get a 1:1
    InstLdweights from the bass lowering; the PE keeps per-row-group weights,
    so a reload of the same AP is a no-op that still costs issue slots and
    weight-bus bandwidth. Only elides LDWs with no sync waits/updates; resets
    tracking at any other PE instruction and at block boundaries.
    """
    n_del = 0
    for blk in nc.main_func.blocks:
        cur = {}  # tile_position -> weights key
        keep = []
        for inst in blk.instructions:
            if isinstance(inst, mybir.InstLdweights):
                a = inst.ins[0]
                key = (a.memref, a.offset, str(a.ap), str(a.dtype),
                       inst.perf_mode, inst.is_transpose)
                tp = inst.tile_position
                si = inst.sync_info
                clean = si is None or (not si.on_wait and not si.on_update)
                if clean and cur.get(tp) == key:
                    n_del += 1
                    continue  # drop redundant reload
                cur[tp] = key
            elif isinstance(inst, mybir.InstMatmult):
                pass  # matmuls don't disturb loaded weights of other groups
            elif getattr(inst, "engine", None) == mybir.EngineType.PE:
                cur = {}  # conservative: unknown PE inst invalidates tracking
            keep.append(inst)
        blk.instructions[:] = keep
    return n_del


def np_in_dt(in_dt=None):
    in_dt = in_dt or IN_DT
    if in_dt == "f16":
        return np.float16
    if in_dt == "bf16":
        import ml_dtypes
        return np.dtype(ml_dtypes.bfloat16)
    return np.float32


def my_in_dt(in_dt=None):
    in_dt = in_dt or IN_DT
    return {"f32r": mybir.dt.float32r, "f16": mybir.dt.float16,
            "bf16": mybir.dt.bfloat16}[in_dt]


# ---------------------------------------------------------------- dual ----

def build_dual(out_dt=OUT_DT, s_pos=None, a_bufs=A_BUFS, ps_bufs=PS_BUFS,
               o_bufs=O_BUFS, repeat=1, in_dt=IN_DT, wpair=None,
               ogran=None, evac=None, no_out=False, no_evac=False, tiles=2,
               static_in=False, m_cols=COUT, order="seq", elide=ELIDE,
               trim=TRIM, psmerge=PSMERGE, in_chunks=None):
    if psmerge and ps_bufs > 2:
        ps_bufs = 2  # two 2-bank tiles per tag x two tags = all 8 PSUM banks
    if in_chunks is None:
        in_chunks = IN_CHUNKS
    s_pos = s_pos or S
    wpair = wpair or WPAIR
    ogran = ogran or OGRAN
    evac = evac or EVAC
    f32 = mybir.dt.float32
    idt = my_in_dt(in_dt)
    out_mydt = f32 if out_dt == "f32" else mybir.dt.float16

    assert L % s_pos == 0 and s_pos % WIN == 0
    n_slabs = L // s_pos
    n_win = s_pos // WIN           # windows per slab (per image)
    assert n_win % ogran == 0

    nc = bacc.Bacc("TRN2", target_bir_lowering=False, debug=False)
    # rows 0:64 image0 channels, rows 64:128 image1 channels
    xt2 = nc.declare_dram_parameter("xt2", [128, XT2_W], idt, isOutput=False)
    # tap j weights replicated on both partition halves
    wt2 = nc.declare_dram_parameter("wt2", [128, 9 * COUT], idt, isOutput=False)
    # cols 0:L image0, cols L:2L image1
    yt = nc.declare_dram_parameter("yt", [COUT, 2 * L], out_mydt, isOutput=True)

    with TileContext(nc) as tc:
        with (
            tc.tile_pool(name="wpool", bufs=1) as wpool,
            tc.tile_pool(name="apool", bufs=a_bufs) as apool,
            tc.tile_pool(name="opool", bufs=o_bufs) as opool,
            tc.tile_pool(name="pspool", bufs=ps_bufs, space="PSUM") as pspool,
        ):
            w_sb = wpool.tile([128, 9 * COUT], idt)
            nc.sync.dma_start(out=w_sb[:, :], in_=wt2[:, :])
            if static_in:
                xd_static = wpool.tile([128, s_pos + MARGIN], idt,
                                       name="xd_static")
                nc.sync.dma_start(out=xd_static[:, :],
                                  in_=xt2[:, 0:s_pos + MARGIN])

            loop_cm = tc.For_i(0, repeat, 1) if repeat > 1 \
                else contextlib.nullcontext()
            ev = 0
            with loop_cm:
              for si in range(n_slabs):
                base = si * s_pos
                if static_in:
                    xd = xd_static
                else:
                    xd = apool.tile([128, s_pos + MARGIN], idt, tag="xd",
                                    name=f"xd_{si}")
                    step = s_pos // in_chunks
                    for k in range(in_chunks):
                        c0 = k * step
                        c1 = (s_pos + MARGIN if k == in_chunks - 1
                              else (k + 1) * step)
                        nc.sync.dma_start(out=xd[:, c0:c1],
                                          in_=xt2[:, base + c0:base + c1])
                for og in range(0, n_win, ogran):
                    sts = [opool.tile([128, ogran * WIN], out_mydt,
                                      tag=f"st{t}", name=f"st{t}_{si}_{og}")
                           for t in range(2)]
                    for wp in range(og, og + ogran, wpair):
                        wn = min(wpair, og + ogran - wp)
                        if psmerge:
                            assert wn == wpair == 2
                            acc2 = [pspool.tile([128, 2 * WIN], f32,
                                                tag=f"acc2_{t}",
                                                name=f"acc2_{si}_{t}_{wp}")
                                    for t in range(2)]
                            accs = [[acc2[t][:, wi * WIN:(wi + 1) * WIN]
                                     for wi in range(wn)] for t in range(2)]
                        else:
                            accs = [[pspool.tile([128, WIN], f32, tag="acc",
                                                 name=f"acc_{si}_{t}_{wi}")
                                     for wi in range(wp, wp + wn)]
                                    for t in range(2)]
                        for j in range(9):
                            r, s = TAPS[j]
                            off = r * W + s
                            st_j = (j == 0)
                            sp_j = (j == 8)
                            if order == "alt":
                                tw = [(t, wi) for wi in range(wn)
                                      for t in range(tiles)]
                            else:
                                tw = [(t, wi) for t in range(tiles)
                                      for wi in range(wn)]
                            for t, wi in tw:
                                p0 = t * 64
                                q0 = (wp + wi) * WIN + off
                                n_mm = WIN
                                if trim and si * n_win + wp + wi == LAST_WIN:
                                    n_mm = LAST_N
                                nc.tensor.matmul(
                                    accs[t][wi][0:m_cols, 0:n_mm],
                                    w_sb[p0:p0 + 64,
                                         j * COUT:j * COUT + m_cols],
                                    xd[p0:p0 + 64, q0:q0 + n_mm],
                                    start=st_j, stop=sp_j,
                                )
                        # evacuate PSUM -> SBUF (cast), alternating engines
                        if no_evac:
                            continue
                        if psmerge:
                            pieces = [(t, acc2[t][:, :], 2 * WIN)
                                      for t in range(tiles)]
                        else:
                            pieces = [(t, accs[t][wi][:], WIN, wi)
                                      for t in range(tiles)
                                      for wi in range(wn)]
                        for piece in pieces:
                            if psmerge:
                                t, src, width = piece
                                c0 = (wp - og) * WIN
                            else:
                                t, src, width, wi = piece
                                c0 = (wp - og + wi) * WIN
                            dst = sts[t][:, c0:c0 + width]
                            if evac == "v" or (evac == "vs" and ev % 2 == 0):
                                nc.vector.tensor_copy(dst, src)
                            else:
                                nc.scalar.activation(
                                    dst, src,
                                    mybir.ActivationFunctionType.Copy)
                            ev += 1
                    if no_out or no_evac:
                        continue
                    for t in range(tiles):
                        col = t * L + base + og * WIN
                        nc.sync.dma_start(
                            out=yt[:, col:col + ogran * WIN],
                            in_=sts[t][:, :])
    if elide:
        elide_redundant_ldweights(nc)
    nc.compile()
    return nc


def prep_xt2(xs, in_dt=None):
    """xs: (2, H, W, 64) f32 -> (128, XT2_W): img0 chans on rows 0:64."""
    out = np.zeros((128, XT2_W), dtype=np_in_dt(in_dt))
    for t in range(N_IMG):
        flat = np.ascontiguousarray(xs[t].transpose(2, 0, 1)).reshape(CIN, L)
        out[t * CIN:(t + 1) * CIN, :L] = flat
    return out


def pack_wt2(w, in_dt=None):
    """w: (3,3,64,128) -> (128, 9*128), tap j on both partition halves."""
    wt = np.zeros((128, 9 * COUT), dtype=np_in_dt(in_dt))
    for j, (r, s) in enumerate(TAPS):
        wt[0:CIN, j * COUT:(j + 1) * COUT] = w[r, s]
        wt[CIN:128, j * COUT:(j + 1) * COUT] = w[r, s]
    return wt


def post_yt_dual(yt_arr):
    """(128, 2L) -> (2, 222, 222, 128) f32."""
    y = np.asarray(yt_arr, dtype=np.float32).reshape(COUT, N_IMG, H, W)
    y = y[:, :, :H - 2, :W - 2]
    return np.ascontiguousarray(y.transpose(1, 2, 3, 0))


# ------------------------------------------------------------ v2 (old) ----

V2_S = 4096
V2_XT_W = N_IMG * L + WIN


def make_plan_v2():
    return ([(0, r * W, 128, [(r, 0), (r, 1)]) for r in range(3)]
            + [(1, 2, 128, [(0, 2), (1, 2)]),
               (0, 2 * W + 2, 64, [(2, 2), None])])


def build_v2(out_dt=OUT_DT, s_pos=V2_S, a_bufs=A_BUFS, ps_bufs=PS_BUFS,
             o_bufs=8, repeat=1, in_dt=IN_DT):
    plan = make_plan_v2()
    n_mm = len(plan)
    Q = N_IMG * L
    f32 = mybir.dt.float32
    idt = my_in_dt(in_dt)
    out_mydt = f32 if out_dt == "f32" else mybir.dt.float16

    nc = bacc.Bacc("TRN2", target_bir_lowering=False, debug=False)
    xt = nc.declare_dram_parameter("xt", [CIN, V2_XT_W], idt, isOutput=False)
    wt = nc.declare_dram_parameter("wt", [n_mm, 128, COUT], idt, isOutput=False)
    yt = nc.declare_dram_parameter("yt", [COUT, Q], out_mydt, isOutput=True)

    with TileContext(nc) as tc:
        with (
            tc.tile_pool(name="wpool", bufs=1) as wpool,
            tc.tile_pool(name="apool", bufs=a_bufs) as apool,
            tc.tile_pool(name="opool", bufs=o_bufs) as opool,
            tc.tile_pool(name="pspool", bufs=ps_bufs, space="PSUM") as pspool,
        ):
            w_sb = wpool.tile([128, n_mm * COUT], idt)
            for i in range(n_mm):
                nc.sync.dma_start(out=w_sb[:, i * COUT:(i + 1) * COUT],
                                  in_=wt[i, :, :])

            n_slabs = (Q + s_pos - 1) // s_pos
            loop_cm = tc.For_i(0, repeat, 1) if repeat > 1 \
                else contextlib.nullcontext()
            with loop_cm:
              for si in range(n_slabs):
                base = si * s_pos
                sh = min(s_pos, Q - base)
                tiles = [apool.tile([128, s_pos + MARGIN], idt, tag=f"t{t}",
                                    name=f"tile{t}_{si}")
                         for t in range(2)]
                # all four halves straight from HBM (no SBUF->SBUF chains)
                nc.sync.dma_start(out=tiles[0][0:CIN, 0:sh + MARGIN],
                                  in_=xt[:, base:base + sh + MARGIN])
                nc.sync.dma_start(out=tiles[0][CIN:128, 0:2 * W + sh],
                                  in_=xt[:, base + 1:base + 1 + 2 * W + sh])
                nc.sync.dma_start(out=tiles[1][0:CIN, 0:sh + 2],
                                  in_=xt[:, base:base + sh + 2])
                nc.sync.dma_start(out=tiles[1][CIN:128, 0:sh + 2],
                                  in_=xt[:, base + W:base + W + sh + 2])

                for q0 in range(0, sh, WIN):
                    acc = pspool.tile([128, WIN], f32)
                    for j, (t, off, kk, _) in enumerate(plan):
                        nc.tensor.matmul(
                            acc[:],
                            w_sb[0:kk, j * COUT:(j + 1) * COUT],
                            tiles[t][0:kk, off + q0: off + q0 + WIN],
                            start=(j == 0),
                            stop=(j == n_mm - 1),
                        )
                    st = opool.tile([128, WIN], out_mydt)
                    nc.vector.tensor_copy(st[:], acc[:])
                    nc.sync.dma_start(out=yt[:, base + q0: base + q0 + WIN],
                                      in_=st[:])
    nc.compile()
    return nc


def pack_wt_v2(w, in_dt=None):
    plan = make_plan_v2()
    wt = np.zeros((len(plan), 128, COUT), dtype=np_in_dt(in_dt))
    for i, (_, _, _, taps) in enumerate(plan):
        (r0, s0), bot = taps
        wt[i, 0:CIN] = w[r0, s0]
        if bot is not None:
            r1, s1 = bot
            wt[i, CIN:128] = w[r1, s1]
    return wt


def prep_xt_v2(xs, in_dt=None):
    flat = np.ascontiguousarray(xs.transpose(3, 0, 1, 2)).reshape(CIN, N_IMG * L)
    out = np.zeros((CIN, V2_XT_W), dtype=np_in_dt(in_dt))
    out[:, :flat.shape[1]] = flat
    return out


def post_yt_v2(yt_arr):
    y = np.asarray(yt_arr, dtype=np.float32).reshape(COUT, N_IMG, H, W)
    y = y[:, :, :H - 2, :W - 2]
    return np.ascontiguousarray(y.transpose(1, 2, 3, 0))


# ------------------------------------------------------------ driver ------

def build_nc(repeat=1, variant=None):
    variant = variant or VARIANT
    if variant == "dual":
        return build_dual(repeat=repeat)
    return build_v2(repeat=repeat)


_NC_CACHE = {}


def _get_nc():
    key = (VARIANT, OUT_DT, IN_DT, S, WPAIR, A_BUFS, PS_BUFS, O_BUFS, OGRAN,
           EVAC)
    if key not in _NC_CACHE:
        _NC_CACHE[key] = build_nc()
    return _NC_CACHE[key]


def make_in_maps(x, w, variant=None):
    variant = variant or VARIANT
    if variant == "dual":
        wt = pack_wt2(w)
        return [{"xt2": prep_xt2(x[c * N_IMG:(c + 1) * N_IMG]), "wt2": wt}
                for c in range(N_CORES)]
    wt = pack_wt_v2(w)
    return [{"xt": prep_xt_v2(x[c * N_IMG:(c + 1) * N_IMG]), "wt": wt}
            for c in range(N_CORES)]


def kernel(x, w):
    x = np.asarray(x, dtype=np.float32)
    w = np.asarray(w, dtype=np.float32)
    nc = _get_nc()
    in_maps = make_in_maps(x, w)
    res = run_bass_kernel_spmd(nc, in_maps, list(range(N_CORES)))
    out = np.empty((N_CORES * N_IMG, H - 2, W - 2, COUT), dtype=np.float32)
    post = post_yt_dual if VARIANT == "dual" else post_yt_v2
    for c in range(N_CORES):
        out[c * N_IMG:(c + 1) * N_IMG] = post(res.results[c]["yt"])
    return out



# revision 20
# speedup vs baseline: 1.0540x; 1.0400x over previous
"""Conv2d 3x3 VALID (NHWC x HWIO -> NHWC) on 8 Trainium2 NeuronCores.

Strategy ("dual"): data-parallel over batch (2 images/core), and within a
core the two images run concurrently on the two 64-row tiles of the PE
array (64x128 row-tiling mode):

  - SBUF partitions 0:64  hold image 0's 64 channels (flat H*W signal)
  - SBUF partitions 64:128 hold image 1's 64 channels
  - PE tile T0 (rows 0:64)  computes image 0, tile T8 (rows 64:128) image 1

Per 512-position window each image needs 9 K=64 matmuls (one per conv tap,
each just a different column offset into the same SBUF slab - no shifted
data copies), accumulated in that image's own PSUM bank.  The two tiles run
concurrently, so the effective cost is 9 x 512 cycles per *two* windows =
4.5 K=128-equivalent matmuls per window (the dense-packing floor).

Outputs at flat positions whose column lands in {W-2, W-1} or row in
{H-2, H-1} are garbage and are sliced away host-side.

Self-contained: hardcodes shapes from the problem spec
  x: (16, 224, 224, 64) f32, w: (3, 3, 64, 128) f32 -> y: (16, 222, 222, 128).
"""
import contextlib
import os
import numpy as np

import concourse.bacc as bacc
import concourse.mybir as mybir
from concourse.tile import TileContext
from concourse.bass_utils import run_bass_kernel_spmd

N_CORES = 8
N_IMG = 2          # images per core
H = W = 224
CIN, COUT = 64, 128
L = H * W          # 50176 flat positions per image
WIN = 512          # window width (one fp32 PSUM bank)
MARGIN = 2 * W + 4
XT2_W = L + WIN    # zero-padded per-image input width

VARIANT = os.environ.get("CONV_VARIANT", "dual")
OUT_DT = os.environ.get("CONV_OUT_DT", "f16")
IN_DT = os.environ.get("CONV_IN_DT", "f16")
S = int(os.environ.get("CONV_S", "7168"))        # slab positions (dual)
WPAIR = int(os.environ.get("CONV_WPAIR", "2"))   # windows per tap-group
A_BUFS = int(os.environ.get("CONV_A_BUFS", "4"))
PS_BUFS = int(os.environ.get("CONV_PS_BUFS", "8"))
O_BUFS = int(os.environ.get("CONV_O_BUFS", "3"))
OGRAN = int(os.environ.get("CONV_OGRAN", "14"))  # windows per output DMA
EVAC = os.environ.get("CONV_EVAC", "vs")         # v, s, or vs (alternate)
ELIDE = os.environ.get("CONV_ELIDE", "1") == "1"
TRIM = os.environ.get("CONV_TRIM", "1") == "1"
PSMERGE = os.environ.get("CONV_PSMERGE", "1") == "1"
IN_CHUNKS = int(os.environ.get("CONV_INCH", "4"))
OUT_CHUNKS = int(os.environ.get("CONV_OUTCH", "2"))
# last window holding any valid output: flat pos 49664..49725 -> N=64 suffices
LAST_WIN = (H * (H - 2)) // WIN  # 97
LAST_N = 64

TAPS = [(r, s) for r in range(3) for s in range(3)]


def elide_redundant_ldweights(nc):
    """Remove InstLdweights that reload the identical weights already resident
    in the same PE row-group (tile_position). Run before nc.compile().

    Consecutive matmuls that reuse the same stationary operand each get a 1:1
    InstLdweights from the bass lowering; the PE keeps per-row-group weights,
    so a reload of the same AP is a no-op that still costs issue slots and
    weight-bus bandwidth. Only elides LDWs with no sync waits/updates; resets
    tracking at any other PE instruction and at block boundaries.
    """
    n_del = 0
    for blk in nc.main_func.blocks:
        cur = {}  # tile_position -> weights key
        keep = []
        for inst in blk.instructions:
            if isinstance(inst, mybir.InstLdweights):
                a = inst.ins[0]
                key = (a.memref, a.offset, str(a.ap), str(a.dtype),
                       inst.perf_mode, inst.is_transpose)
                tp = inst.tile_position
                si = inst.sync_info
                clean = si is None or (not si.on_wait and not si.on_update)
                if clean and cur.get(tp) == key:
                    n_del += 1
                    continue  # drop redundant reload
                cur[tp] = key
            elif isinstance(inst, mybir.InstMatmult):
                pass  # matmuls don't disturb loaded weights of other groups
            elif getattr(inst, "engine", None) == mybir.EngineType.PE:
                cur = {}  # conservative: unknown PE inst invalidates tracking
            keep.append(inst)
        blk.instructions[:] = keep
    return n_del


def np_in_dt(in_dt=None):
    in_dt = in_dt or IN_DT
    if in_dt == "f16":
        return np.float16
    if in_dt == "bf16":
        import ml_dtypes
        return np.dtype(ml_dtypes.bfloat16)
    return np.float32


def my_in_dt(in_dt=None):
    in_dt = in_dt or IN_DT
    return {"f32r": mybir.dt.float32r, "f16": mybir.dt.float16,
            "bf16": mybir.dt.bfloat16}[in_dt]


# ---------------------------------------------------------------- dual ----

def build_dual(out_dt=OUT_DT, s_pos=None, a_bufs=A_BUFS, ps_bufs=PS_BUFS,
               o_bufs=O_BUFS, repeat=1, in_dt=IN_DT, wpair=None,
               ogran=None, evac=None, no_out=False, no_evac=False, tiles=2,
               static_in=False, m_cols=COUT, order="seq", elide=ELIDE,
               trim=TRIM, psmerge=PSMERGE, in_chunks=None, out_chunks=None):
    if psmerge and ps_bufs > 2:
        ps_bufs = 2  # two 2-bank tiles per tag x two tags = all 8 PSUM banks
    if in_chunks is None:
        in_chunks = IN_CHUNKS
    if out_chunks is None:
        out_chunks = OUT_CHUNKS
    s_pos = s_pos or S
    wpair = wpair or WPAIR
    ogran = ogran or OGRAN
    evac = evac or EVAC
    f32 = mybir.dt.float32
    idt = my_in_dt(in_dt)
    out_mydt = f32 if out_dt == "f32" else mybir.dt.float16

    assert L % s_pos == 0 and s_pos % WIN == 0
    n_slabs = L // s_pos
    n_win = s_pos // WIN           # windows per slab (per image)
    assert n_win % ogran == 0

    nc = bacc.Bacc("TRN2", target_bir_lowering=False, debug=False)
    # rows 0:64 image0 channels, rows 64:128 image1 channels
    xt2 = nc.declare_dram_parameter("xt2", [128, XT2_W], idt, isOutput=False)
    # tap j weights replicated on both partition halves
    wt2 = nc.declare_dram_parameter("wt2", [128, 9 * COUT], idt, isOutput=False)
    # cols 0:L image0, cols L:2L image1
    yt = nc.declare_dram_parameter("yt", [COUT, 2 * L], out_mydt, isOutput=True)

    with TileContext(nc) as tc:
        with (
            tc.tile_pool(name="wpool", bufs=1) as wpool,
            tc.tile_pool(name="apool", bufs=a_bufs) as apool,
            tc.tile_pool(name="opool", bufs=o_bufs) as opool,
            tc.tile_pool(name="pspool", bufs=ps_bufs, space="PSUM") as pspool,
        ):
            w_sb = wpool.tile([128, 9 * COUT], idt)
            nc.sync.dma_start(out=w_sb[:, :], in_=wt2[:, :])
            if static_in:
                xd_static = wpool.tile([128, s_pos + MARGIN], idt,
                                       name="xd_static")
                nc.sync.dma_start(out=xd_static[:, :],
                                  in_=xt2[:, 0:s_pos + MARGIN])

            loop_cm = tc.For_i(0, repeat, 1) if repeat > 1 \
                else contextlib.nullcontext()
            ev = 0
            with loop_cm:
              for si in range(n_slabs):
                base = si * s_pos
                if static_in:
                    xd = xd_static
                else:
                    xd = apool.tile([128, s_pos + MARGIN], idt, tag="xd",
                                    name=f"xd_{si}")
                    step = s_pos // in_chunks
                    for k in range(in_chunks):
                        c0 = k * step
                        c1 = (s_pos + MARGIN if k == in_chunks - 1
                              else (k + 1) * step)
                        nc.sync.dma_start(out=xd[:, c0:c1],
                                          in_=xt2[:, base + c0:base + c1])
                for og in range(0, n_win, ogran):
                    sts = [opool.tile([128, ogran * WIN], out_mydt,
                                      tag=f"st{t}", name=f"st{t}_{si}_{og}")
                           for t in range(2)]
                    for wp in range(og, og + ogran, wpair):
                        wn = min(wpair, og + ogran - wp)
                        if psmerge:
                            assert wn == wpair == 2
                            acc2 = [pspool.tile([128, 2 * WIN], f32,
                                                tag=f"acc2_{t}",
                                                name=f"acc2_{si}_{t}_{wp}")
                                    for t in range(2)]
                            accs = [[acc2[t][:, wi * WIN:(wi + 1) * WIN]
                                     for wi in range(wn)] for t in range(2)]
                        else:
                            accs = [[pspool.tile([128, WIN], f32, tag="acc",
                                                 name=f"acc_{si}_{t}_{wi}")
                                     for wi in range(wp, wp + wn)]
                                    for t in range(2)]
                        for j in range(9):
                            r, s = TAPS[j]
                            off = r * W + s
                            st_j = (j == 0)
                            sp_j = (j == 8)
                            if order == "alt":
                                tw = [(t, wi) for wi in range(wn)
                                      for t in range(tiles)]
                            else:
                                tw = [(t, wi) for t in range(tiles)
                                      for wi in range(wn)]
                            for t, wi in tw:
                                p0 = t * 64
                                q0 = (wp + wi) * WIN + off
                                n_mm = WIN
                                if trim and si * n_win + wp + wi == LAST_WIN:
                                    n_mm = LAST_N
                                nc.tensor.matmul(
                                    accs[t][wi][0:m_cols, 0:n_mm],
                                    w_sb[p0:p0 + 64,
                                         j * COUT:j * COUT + m_cols],
                                    xd[p0:p0 + 64, q0:q0 + n_mm],
                                    start=st_j, stop=sp_j,
                                )
                        # evacuate PSUM -> SBUF (cast), alternating engines
                        if no_evac:
                            continue
                        if psmerge:
                            pieces = [(t, acc2[t][:, :], 2 * WIN)
                                      for t in range(tiles)]
                        else:
                            pieces = [(t, accs[t][wi][:], WIN, wi)
                                      for t in range(tiles)
                                      for wi in range(wn)]
                        for piece in pieces:
                            if psmerge:
                                t, src, width = piece
                                c0 = (wp - og) * WIN
                            else:
                                t, src, width, wi = piece
                                c0 = (wp - og + wi) * WIN
                            dst = sts[t][:, c0:c0 + width]
                            if evac == "v" or (evac == "vs" and ev % 2 == 0):
                                nc.vector.tensor_copy(dst, src)
                            else:
                                nc.scalar.activation(
                                    dst, src,
                                    mybir.ActivationFunctionType.Copy)
                            ev += 1
                    if no_out or no_evac:
                        continue
                    for t in range(tiles):
                        col = t * L + base + og * WIN
                        ow = ogran * WIN
                        ostep = ow // out_chunks
                        for c in range(out_chunks):
                            nc.sync.dma_start(
                                out=yt[:, col + c * ostep:
                                       col + (c + 1) * ostep],
                                in_=sts[t][:, c * ostep:(c + 1) * ostep])
    if elide:
        elide_redundant_ldweights(nc)
    nc.compile()
    return nc


def prep_xt2(xs, in_dt=None):
    """xs: (2, H, W, 64) f32 -> (128, XT2_W): img0 chans on rows 0:64."""
    out = np.zeros((128, XT2_W), dtype=np_in_dt(in_dt))
    for t in range(N_IMG):
        flat = np.ascontiguousarray(xs[t].transpose(2, 0, 1)).reshape(CIN, L)
        out[t * CIN:(t + 1) * CIN, :L] = flat
    return out


def pack_wt2(w, in_dt=None):
    """w: (3,3,64,128) -> (128, 9*128), tap j on both partition halves."""
    wt = np.zeros((128, 9 * COUT), dtype=np_in_dt(in_dt))
    for j, (r, s) in enumerate(TAPS):
        wt[0:CIN, j * COUT:(j + 1) * COUT] = w[r, s]
        wt[CIN:128, j * COUT:(j + 1) * COUT] = w[r, s]
    return wt


def post_yt_dual(yt_arr):
    """(128, 2L) -> (2, 222, 222, 128) f32."""
    y = np.asarray(yt_arr, dtype=np.float32).reshape(COUT, N_IMG, H, W)
    y = y[:, :, :H - 2, :W - 2]
    return np.ascontiguousarray(y.transpose(1, 2, 3, 0))


# ------------------------------------------------------------ v2 (old) ----

V2_S = 4096
V2_XT_W = N_IMG * L + WIN


def make_plan_v2():
    return ([(0, r * W, 128, [(r, 0), (r, 1)]) for r in range(3)]
            + [(1, 2, 128, [(0, 2), (1, 2)]),
               (0, 2 * W + 2, 64, [(2, 2), None])])


def build_v2(out_dt=OUT_DT, s_pos=V2_S, a_bufs=A_BUFS, ps_bufs=PS_BUFS,
             o_bufs=8, repeat=1, in_dt=IN_DT):
    plan = make_plan_v2()
    n_mm = len(plan)
    Q = N_IMG * L
    f32 = mybir.dt.float32
    idt = my_in_dt(in_dt)
    out_mydt = f32 if out_dt == "f32" else mybir.dt.float16

    nc = bacc.Bacc("TRN2", target_bir_lowering=False, debug=False)
    xt = nc.declare_dram_parameter("xt", [CIN, V2_XT_W], idt, isOutput=False)
    wt = nc.declare_dram_parameter("wt", [n_mm, 128, COUT], idt, isOutput=False)
    yt = nc.declare_dram_parameter("yt", [COUT, Q], out_mydt, isOutput=True)

    with TileContext(nc) as tc:
        with (
            tc.tile_pool(name="wpool", bufs=1) as wpool,
            tc.tile_pool(name="apool", bufs=a_bufs) as apool,
            tc.tile_pool(name="opool", bufs=o_bufs) as opool,
            tc.tile_pool(name="pspool", bufs=ps_bufs, space="PSUM") as pspool,
        ):
            w_sb = wpool.tile([128, n_mm * COUT], idt)
            for i in range(n_mm):
                nc.sync.dma_start(out=w_sb[:, i * COUT:(i + 1) * COUT],
                                  in_=wt[i, :, :])

            n_slabs = (Q + s_pos - 1) // s_pos
            loop_cm = tc.For_i(0, repeat, 1) if repeat > 1 \
                else contextlib.nullcontext()
            with loop_cm:
              for si in range(n_slabs):
                base = si * s_pos
                sh = min(s_pos, Q - base)
                tiles = [apool.tile([128, s_pos + MARGIN], idt, tag=f"t{t}",
                                    name=f"tile{t}_{si}")
                         for t in range(2)]
                # all four halves straight from HBM (no SBUF->SBUF chains)
                nc.sync.dma_start(out=tiles[0][0:CIN, 0:sh + MARGIN],
                                  in_=xt[:, base:base + sh + MARGIN])
                nc.sync.dma_start(out=tiles[0][CIN:128, 0:2 * W + sh],
                                  in_=xt[:, base + 1:base + 1 + 2 * W + sh])
                nc.sync.dma_start(out=tiles[1][0:CIN, 0:sh + 2],
                                  in_=xt[:, base:base + sh + 2])
                nc.sync.dma_start(out=tiles[1][CIN:128, 0:sh + 2],
                                  in_=xt[:, base + W:base + W + sh + 2])

                for q0 in range(0, sh, WIN):
                    acc = pspool.tile([128, WIN], f32)
                    for j, (t, off, kk, _) in enumerate(plan):
                        nc.tensor.matmul(
                            acc[:],
                            w_sb[0:kk, j * COUT:(j + 1) * COUT],
                            tiles[t][0:kk, off + q0: off + q0 + WIN],
                            start=(j == 0),
                            stop=(j == n_mm - 1),
                        )
                    st = opool.tile([128, WIN], out_mydt)
                    nc.vector.tensor_copy(st[:], acc[:])
                    nc.sync.dma_start(out=yt[:, base + q0: base + q0 + WIN],
                                      in_=st[:])
    nc.compile()
    return nc


def pack_wt_v2(w, in_dt=None):
    plan = make_plan_v2()
    wt = np.zeros((len(plan), 128, COUT), dtype=np_in_dt(in_dt))
    for i, (_, _, _, taps) in enumerate(plan):
        (r0, s0), bot = taps
        wt[i, 0:CIN] = w[r0, s0]
        if bot is not None:
            r1, s1 = bot
            wt[i, CIN:128] = w[r1, s1]
    return wt


def prep_xt_v2(xs, in_dt=None):
    flat = np.ascontiguousarray(xs.transpose(3, 0, 1, 2)).reshape(CIN, N_IMG * L)
    out = np.zeros((CIN, V2_XT_W), dtype=np_in_dt(in_dt))
    out[:, :flat.shape[1]] = flat
    return out


def post_yt_v2(yt_arr):
    y = np.asarray(yt_arr, dtype=np.float32).reshape(COUT, N_IMG, H, W)
    y = y[:, :, :H - 2, :W - 2]
    return np.ascontiguousarray(y.transpose(1, 2, 3, 0))


# ------------------------------------------------------------ driver ------

def build_nc(repeat=1, variant=None):
    variant = variant or VARIANT
    if variant == "dual":
        return build_dual(repeat=repeat)
    return build_v2(repeat=repeat)


_NC_CACHE = {}


def _get_nc():
    key = (VARIANT, OUT_DT, IN_DT, S, WPAIR, A_BUFS, PS_BUFS, O_BUFS, OGRAN,
           EVAC)
    if key not in _NC_CACHE:
        _NC_CACHE[key] = build_nc()
    return _NC_CACHE[key]


def make_in_maps(x, w, variant=None):
    variant = variant or VARIANT
    if variant == "dual":
        wt = pack_wt2(w)
        return [{"xt2": prep_xt2(x[c * N_IMG:(c + 1) * N_IMG]), "wt2": wt}
                for c in range(N_CORES)]
    wt = pack_wt_v2(w)
    return [{"xt": prep_xt_v2(x[c * N_IMG:(c + 1) * N_IMG]), "wt": wt}
            for c in range(N_CORES)]


def kernel(x, w):
    x = np.asarray(x, dtype=np.float32)
    w = np.asarray(w, dtype=np.float32)
    nc = _get_nc()
    in_maps = make_in_maps(x, w)
    res = run_bass_kernel_spmd(nc, in_maps, list(range(N_CORES)))
    out = np.empty((N_CORES * N_IMG, H - 2, W - 2, COUT), dtype=np.float32)
    post = post_yt_dual if VARIANT == "dual" else post_yt_v2
    for c in range(N_CORES):
        out[c * N_IMG:(c + 1) * N_IMG] = post(res.results[c]["yt"])
    return out

